# revision 35
# baseline (speedup 1.0000x reference)
"""GQA attention kernel for Trainium2, 8-core tensor-parallel over kv heads.

Reference computation (fp32):
  q  = query @ q_proj.T + q_bias      -> heads (g-major): dq = gi*H*D + hi*D + d
  kv = query @ kv_proj.T + kv_bias    -> per kv head hi: k = cols [hi*2D, hi*2D+D), v = next D
  attn = softmax(q k^T / sqrt(D));  out = (attn v) @ out_proj.T + out_bias

Sharding: 8 cores; core c handles kv head h0 = c//2 and 4 query-head groups
gis = [0..3] (c even) or [4..7] (c odd). Each core computes a full-shape
partial of the output (rank-256 contribution); host sums the 8 partials.

On-core dataflow (bf16 matmuls, fp32 PSUM accumulation; t = n*L + l):
  Software pipeline over the batch dim n:
    p1(n0) -> [p2(n0) || p1(n1)] -> [p2(n1) || p3(n0)] -> p3(n1)
  p2 is ACT-bound (exp of the full score matrix); the interleaved p1/p3
  matmuls fill the Tensor-engine slack underneath it.
  p2 inner loop per (pair, lc512, mc): sA/sB score MMs (K=64, row-tiled
  quadrants T0/T8), one exp ACTIVATE over [sA|sB] (N=1024), two AV MMs
  accumulating [attnout^T; denom] via ones-augmented V'.
  PSUM: 2 score slots (4 banks) + AV pair (2) + 2 interleave banks = 8.
"""
import sys

sys.path.insert(0, "/opt/trn_rl_repo")

import ml_dtypes
import numpy as np

import concourse.bass as bass
import concourse.mybir as mybir
import concourse.tile as tile
from concourse import bacc

H, G, D = 4, 8, 64
L, N, E = 2048, 2, 2048
T = N * L
P = 128
DQ = 256  # per-core q dim: 4 groups x 64
SCALE = float(D) ** -0.5
F32 = mybir.dt.float32
BF16 = mybir.dt.bfloat16
ADD = mybir.AluOpType.add
EXP = mybir.ActivationFunctionType.Exp
IDENT = mybir.ActivationFunctionType.Identity


def pbcast(ap2d, p):
    """[1, F] AP -> [p, F] AP broadcast across partitions (stride 0)."""
    return bass.AP(tensor=ap2d.tensor, offset=ap2d.offset, ap=[[0, p]] + list(ap2d.ap[1:]))


def drain(gen):
    if gen is not None:
        for _ in gen:
            pass


def build_nc():
    nc = bacc.Bacc("TRN2", target_bir_lowering=False, debug=False)

    qT = nc.dram_tensor("qT", [E, T], BF16, kind="ExternalInput").ap()
    qpT = nc.dram_tensor("qpT", [E, DQ], BF16, kind="ExternalInput").ap()
    kvpT = nc.dram_tensor("kvpT", [E, P], BF16, kind="ExternalInput").ap()
    opT = nc.dram_tensor("opT", [DQ, E], BF16, kind="ExternalInput").ap()
    qb = nc.dram_tensor("qb", [P, 2], F32, kind="ExternalInput").ap()
    kvb = nc.dram_tensor("kvb", [P, 1], F32, kind="ExternalInput").ap()
    ident = nc.dram_tensor("ident", [P, P], BF16, kind="ExternalInput").ap()
    ones16 = nc.dram_tensor("ones16", [P, 16], BF16, kind="ExternalInput").ap()
    out = nc.dram_tensor("out", [T, E], BF16, kind="ExternalOutput").ap()
    denombuf = nc.dram_tensor("denombuf", [1, 4 * 4096], F32, kind="Internal").ap()
    recipbuf = nc.dram_tensor("recipbuf", [1, 4 * 4096], F32, kind="Internal").ap()

    with tile.TileContext(nc) as tc, tc.tile_pool(name="data", bufs=1) as data, \
            tc.tile_pool(name="consts", bufs=1) as consts:
        # ---- consts / weights (DMA order tuned: critical path first) ----
        # gather weight chunks [ec-block-major] into SBUF with hand-built APs:
        # partition p, block a <- dram row a*128+p.  Order: only what the
        # first p1 matmul needs, then the rest (shortens the DMA lead-in).
        kvpT_sb = data.tile([P, 16 * P], BF16)
        nc.sync.dma_start(
            out=kvpT_sb[:].rearrange("p (a c) -> p a c", c=P),
            in_=bass.AP(tensor=kvpT.tensor, offset=0,
                        ap=[[P, P], [P * P, 16], [1, P]]))
        qpT_sb = data.tile([P, 16 * DQ], BF16)

        def load_qpT(qc):
            nc.sync.dma_start(
                out=qpT_sb[:, qc * 4 * DQ:(qc + 1) * 4 * DQ].rearrange(
                    "p (a c) -> p a c", c=DQ),
                in_=bass.AP(tensor=qpT.tensor, offset=qc * 4 * P * DQ,
                            ap=[[DQ, P], [P * DQ, 4], [1, DQ]]))

        for qc in range(4):
            load_qpT(qc)
        qb_sb = consts.tile([P, 2], F32)
        kvb_sb = consts.tile([P, 1], F32)
        warm = consts.tile([1, 2], BF16)

        QT0 = data.tile([P, T], BF16)  # dq 0:128   (heads A=gi0/B=gi1)
        QT1 = data.tile([P, T], BF16)  # dq 128:256 (heads A=gi2/B=gi3)
        KVT = data.tile([P, T], BF16)  # k rows 0:64, v rows 64:128
        KTdup = data.tile([P, T], BF16)  # k rows duplicated at partitions 64:128
        attn0 = data.tile([P, T], BF16)  # normalized attnout^T c-chunk 0
        attn1 = data.tile([P, T], BF16)  # c-chunk 1
        Vtmp = data.tile([64, T], BF16)
        Vp = [data.tile([P, 16 * 65], BF16, name=f"vp{n}", tag=f"vp{n}") for n in range(N)]
        opT_sb = [data.tile([P, E], BF16, name=f"opt{cc}", tag=f"opt{cc}") for cc in range(2)]
        identb = consts.tile([P, P], BF16)
        sc = data.tile([64, 512], BF16)
        bct = data.tile([P, L], F32)

        def p1_evict(tc_, psq0, psq1, pskv, tcols):
            nc.vector.tensor_scalar(QT0[:, tcols], psq0[:], qb_sb[:, 0:1], None, op0=ADD)
            nc.scalar.activation(QT1[:, tcols], psq1[:], IDENT, bias=qb_sb[:, 1:2])
            nc.vector.tensor_scalar(KVT[:, tcols], pskv[:], kvb_sb[:, 0:1], None, op0=ADD)
            nc.sync.dma_start(out=KTdup[64:128, tcols], in_=KVT[0:64, tcols])
            nc.sync.dma_start(out=Vtmp[0:64, tcols], in_=KVT[64:128, tcols])

        def vp_build(n, psT):
            vcol = Vp[n].rearrange("p (m c) -> p m c", c=65)[:, :, 64:65]
            nc.sync.dma_start(out=vcol, in_=ones16)
            for mc in range(16):
                pt = psT.tile([P, 64], BF16, tag="pt")
                nc.tensor.transpose(pt[:], Vtmp[0:64, n * L + mc * P:n * L + (mc + 1) * P],
                                    identb[0:64, 0:64])
                nc.vector.tensor_copy(Vp[n][:, mc * 65:mc * 65 + 64], pt[:])

        # ---------------- p1(n0): tchunks 0,1 (streaming) ----------------
        with tc.tile_pool(name="qload", bufs=6) as qload, \
                tc.tile_pool(name="ps1", bufs=1, space="PSUM") as ps1:
            for tchunk in range(2):
                tcols = slice(tchunk * 1024, (tchunk + 1) * 1024)
                pq0 = ps1.tile([P, 1024], F32, tag="pq0")
                pq1 = ps1.tile([P, 1024], F32, tag="pq1")
                pkv = ps1.tile([P, 1024], F32, tag="pkv")
                for ec in range(16):
                    qt = qload.tile([P, 1024], BF16, tag="qt")
                    nc.sync.dma_start(out=qt[:], in_=qT[ec * P:(ec + 1) * P, tcols])
                    if tchunk == 0 and ec == 0:
                        nc.sync.dma_start(out=qb_sb[:], in_=qb)
                        nc.sync.dma_start(out=kvb_sb[:], in_=kvb)
                        nc.sync.dma_start(out=identb[:], in_=ident)
                        nc.scalar.activation(warm[:], qb_sb[0:1, 0:2], EXP)
                    first, last = ec == 0, ec == 15
                    for ps_t, w in ((pq0, qpT_sb[:, ec * DQ:ec * DQ + P]),
                                    (pq1, qpT_sb[:, ec * DQ + P:(ec + 1) * DQ]),
                                    (pkv, kvpT_sb[:, ec * P:(ec + 1) * P])):
                        for lq in range(2):
                            nc.tensor.matmul(ps_t[:, lq * 512:(lq + 1) * 512], lhsT=w,
                                             rhs=qt[:, lq * 512:(lq + 1) * 512],
                                             start=first, stop=last)
                p1_evict(tchunk, pq0, pq1, pkv, tcols)

        with tc.tile_pool(name="psT0", bufs=4, space="PSUM") as psT:
            vp_build(0, psT)
        # out-proj weights: needed from p3(n0); loads during p1/p2(n0)
        for cc in range(2):
            nc.sync.dma_start(out=opT_sb[cc][:], in_=opT[cc * P:(cc + 1) * P, :])

        # ---------------- interleaved fillers ----------------
        def p1_gen(ila, ilb, qload2):
            """p1(n1): tchunks 2,3. [128,512] psum chains, DVE-only evictions."""
            il = (ila, ilb)
            for tchunk in range(2, 4):
                tcols = slice(tchunk * 1024, (tchunk + 1) * 1024)
                qts = []
                for ec in range(16):
                    qt = qload2.tile([P, 1024], BF16, tag=f"q{ec}")
                    nc.sync.dma_start(out=qt[:], in_=qT[ec * P:(ec + 1) * P, tcols])
                    qts.append(qt)
                    if ec % 4 == 3:
                        yield
                chains = []
                for lq in range(2):
                    cols5 = slice(tchunk * 1024 + lq * 512, tchunk * 1024 + lq * 512 + 512)
                    chains.append((KVT, cols5, kvb_sb[:, 0:1],
                                   [kvpT_sb[:, ec * P:(ec + 1) * P] for ec in range(16)]))
                    chains.append((QT0, cols5, qb_sb[:, 0:1],
                                   [qpT_sb[:, ec * DQ:ec * DQ + P] for ec in range(16)]))
                    chains.append((QT1, cols5, qb_sb[:, 1:2],
                                   [qpT_sb[:, ec * DQ + P:(ec + 1) * DQ] for ec in range(16)]))
                for ci, (dst, cols5, bias, ws) in enumerate(chains):
                    pchain = il[ci % 2]
                    for ec in range(16):
                        nc.tensor.matmul(pchain[:], lhsT=ws[ec],
                                         rhs=qts[ec][:, (ci // 3) * 512:(ci // 3) * 512 + 512],
                                         start=ec == 0, stop=ec == 15)
                        yield
                    nc.vector.tensor_scalar(dst[:, cols5], pchain[:], bias, None, op0=ADD)
                    yield
                nc.sync.dma_start(out=KTdup[64:128, tcols], in_=KVT[0:64, tcols])
                nc.sync.dma_start(out=Vtmp[0:64, tcols], in_=KVT[64:128, tcols])
                yield

        def p3_gen(ila, ilb, ostage, tts, act_evict=False):
            """p3: out_partial[t,e] = attnT.T @ opT, [128,512] psum chunks.
            act_evict alternates PSUM eviction onto the Scalar engine — only
            for t-rows that execute after the exp stream has finished."""
            il = (ila, ilb)
            for tt in tts:
                trows = slice(tt * P, (tt + 1) * P)
                ost = ostage.tile([P, E], BF16, tag="ost")
                for ch in range(4):
                    po = il[ch % 2]
                    ecols = slice(ch * 512, (ch + 1) * 512)
                    nc.tensor.matmul(po[:], lhsT=attn0[:, trows],
                                     rhs=opT_sb[0][:, ecols], start=True, stop=False)
                    yield
                    nc.tensor.matmul(po[:], lhsT=attn1[:, trows],
                                     rhs=opT_sb[1][:, ecols], start=False, stop=True)
                    yield
                    if act_evict:
                        nc.scalar.copy(ost[:, ecols], po[:])
                    else:
                        nc.vector.tensor_copy(ost[:, ecols], po[:])
                    yield
                nc.sync.dma_start(out=out[trows, :], in_=ost[:])
                yield

        # ---------------- p2: attention for batch n ----------------
        def p2_phase(n, filler):
            with tc.tile_pool(name=f"ps2_{n}", bufs=1, space="PSUM") as ps2, \
                    tc.tile_pool(name=f"il_{n}", bufs=1, space="PSUM") as ilp, \
                    tc.tile_pool(name=f"exp_{n}", bufs=4) as expool, \
                    tc.tile_pool(name=f"scr_{n}", bufs=2) as scratch, \
                    tc.tile_pool(name=f"aux_{n}", bufs=2 if filler == "p3" else 1) as aux:
                slots = [ps2.tile([P, 1024], F32, name=f"slot{i}", tag=f"slot{i}")
                         for i in range(2)]
                avp = ps2.tile([65, 1024], F32, tag="avp")
                ila = ilp.tile([P, 512], F32, tag="ila")
                ilb = ilp.tile([P, 512], F32, tag="ilb")
                gens = []
                if filler == "p1":
                    gens.append(p1_gen(ila, ilb, aux))
                else:
                    gens.append(p3_gen(ila, ilb, aux, list(range(16))))

                def pop_item():
                    while gens:
                        try:
                            next(gens[0])
                            return
                        except StopIteration:
                            gens.pop(0)

                for pair in range(2):
                    QTp = QT0 if pair == 0 else QT1
                    attnp = attn0 if pair == 0 else attn1
                    seg = (n * 2 + pair) * 4096
                    for lc in range(4):
                        lcols = slice(n * L + lc * 512, n * L + (lc + 1) * 512)

                        def scores(mc):
                            mo = n * L + mc * P
                            slot = slots[mc & 1]
                            nc.tensor.matmul(slot[:, 0:512], lhsT=KVT[0:64, mo:mo + P],
                                             rhs=QTp[0:64, lcols])
                            nc.tensor.matmul(slot[:, 512:1024],
                                             lhsT=KTdup[64:128, mo:mo + P],
                                             rhs=QTp[64:128, lcols])

                        scores(0)
                        for mc in range(16):
                            eAB = expool.tile([P, 1024], BF16, tag="eab")
                            nc.scalar.activation(eAB[:], slots[mc & 1][:], EXP, scale=SCALE)
                            if mc < 15:
                                scores(mc + 1)
                            first, last = mc == 0, mc == 15
                            vw = Vp[n][:, mc * 65:mc * 65 + 65]
                            nc.tensor.matmul(avp[:, 0:512], lhsT=vw, rhs=eAB[:, 0:512],
                                             start=first, stop=last)
                            nc.tensor.matmul(avp[:, 512:1024], lhsT=vw, rhs=eAB[:, 512:1024],
                                             start=first, stop=last)
                            pop_item()
                            pop_item()
                        # evict attnout rows + denominator rows
                        nc.vector.tensor_copy(attnp[0:64, lcols], avp[0:64, 0:512])
                        scx = scratch.tile([64, 512], BF16, tag="scx")
                        nc.vector.tensor_copy(scx[:], avp[0:64, 512:1024])
                        nc.sync.dma_start(out=attnp[64:128, lcols], in_=scx[:])
                        dn = scratch.tile([1, 1024], F32, tag="dn")
                        nc.vector.tensor_copy(dn[:], avp[64:65, :])
                        sg2 = seg + lc * 1024
                        nc.sync.dma_start(out=denombuf[0:1, sg2:sg2 + 1024], in_=dn[:])
                        # per-lc normalize chain; hides under the next lc's attention
                        packed = scratch.tile([P, 8], F32, tag="packed")
                        nc.sync.dma_start(
                            out=packed[:],
                            in_=denombuf[0:1, sg2:sg2 + 1024].rearrange(
                                "a (p c) -> (a p) c", p=P))
                        recp = scratch.tile([P, 8], F32, tag="recp")
                        nc.vector.reciprocal(recp[:], packed[:])
                        nc.sync.dma_start(
                            out=recipbuf[0:1, sg2:sg2 + 1024].rearrange(
                                "a (p c) -> (a p) c", p=P),
                            in_=recp[:])
                        for hh in range(2):
                            src = bass.AP(tensor=recipbuf.tensor, offset=sg2 + hh * 512,
                                          ap=[[0, 64], [1, 512]])
                            nc.sync.dma_start(
                                out=bct[hh * 64:hh * 64 + 64, lc * 512:(lc + 1) * 512],
                                in_=src)
                        bcs = slice(lc * 512, (lc + 1) * 512)
                        nc.vector.tensor_mul(attnp[:, lcols], attnp[:, lcols], bct[:, bcs])
                        if n == 1 and pair == 1:
                            # t-rows for this lc now final on both pairs ->
                            # their out-projection tiles can go
                            gens.append(p3_gen(ila, ilb, aux,
                                               [16 + lc * 4 + i for i in range(4)],
                                               act_evict=lc == 3))
                for g in gens:
                    drain(g)

        p2_phase(0, "p1")
        with tc.tile_pool(name="psT1", bufs=4, space="PSUM") as psT:
            vp_build(1, psT)
        p2_phase(1, "p3")

    nc.compile()
    return nc


_NC_CACHE = None


def _get_nc():
    global _NC_CACHE
    if _NC_CACHE is None:
        _NC_CACHE = build_nc()
    return _NC_CACHE


def make_in_maps(query, q_proj, q_bias, kv_proj, kv_bias, out_proj):
    """Host-side sharding. Returns list of 8 per-core input dicts."""
    qT_h = np.ascontiguousarray(
        np.asarray(query, dtype=np.float32).transpose(2, 1, 0).reshape(E, T)
    ).astype(ml_dtypes.bfloat16)
    q_proj = np.asarray(q_proj, dtype=np.float32)
    q_bias = np.asarray(q_bias, dtype=np.float32)
    kv_proj = np.asarray(kv_proj, dtype=np.float32)
    kv_bias = np.asarray(kv_bias, dtype=np.float32)
    out_proj = np.asarray(out_proj, dtype=np.float32)
    ident = np.eye(P, dtype=np.float32)

    in_maps = []
    for c in range(8):
        h0 = c // 2
        gis = range(4) if c % 2 == 0 else range(4, 8)
        rows_q = np.array([gi * (H * D) + h0 * D + d for gi in gis for d in range(D)])
        kv_rows = slice(h0 * 2 * D, (h0 + 1) * 2 * D)
        in_maps.append({
            "qT": qT_h,
            "qpT": np.ascontiguousarray(q_proj[rows_q, :].T).astype(ml_dtypes.bfloat16),
            "kvpT": np.ascontiguousarray(kv_proj[kv_rows, :].T).astype(ml_dtypes.bfloat16),
            "opT": np.ascontiguousarray(out_proj[:, rows_q].T).astype(ml_dtypes.bfloat16),
            "qb": np.ascontiguousarray(q_bias[rows_q].reshape(2, P).T),
            "kvb": np.ascontiguousarray(kv_bias[kv_rows].reshape(P, 1)),
            "ident": ident.astype(ml_dtypes.bfloat16),
            "ones16": np.ones((P, 16), dtype=ml_dtypes.bfloat16),
        })
    return in_maps


def kernel(query, q_proj, q_bias, kv_proj, kv_bias, out_proj, out_bias):
    from concourse.bass_utils import run_bass_kernel_spmd

    nc = _get_nc()
    in_maps = make_in_maps(query, q_proj, q_bias, kv_proj, kv_bias, out_proj)
    res = run_bass_kernel_spmd(nc, in_maps, core_ids=list(range(8)))
    total = np.zeros((T, E), dtype=np.float64)
    for rmap in res.results:
        total += rmap["out"].astype(np.float64)
    total += np.asarray(out_bias, dtype=np.float64)[None, :]
    return np.ascontiguousarray(
        total.reshape(N, L, E).transpose(1, 0, 2)).astype(np.float32)
